# revision 19
# baseline (speedup 1.0000x reference)
"""Trainium2 Bass kernel for nn_DiTBlock (HGRN-attention DiT block).

Sharding: 8 cores = 4 batches x 2 half-sequences (1024 tokens each).
All bitlinear matmuls are exact integer arithmetic: activations quantized
to int8-range integers stored in bf16, ternary weights stored in fp8e4
(exact, half the DMA of bf16). The time recurrence h_t = f_t*h_{t-1} + i_t
runs on the DVE tensor_tensor_scan; the half-sequence boundary carry
crosses cores via one AllGather issued right after the last scan and
hidden under the g-projection matmuls + late adaln chunks.

Schedule (vs the phase-sequential baseline):
 - g-projection + adaln chunks 8..11 moved into the AllGather window.
 - o-stage / wo-matmul / LN2 / MLP are software-pipelined per token-block
   so PE matmuls overlap the DVE/ACT quant chains.
 - GpSimd (Pool engine) takes SBUF-only elementwise work (silu muls in
   the scan prep, modulate adds, residual adds) off the DVE.
 - Per-phase PSUM pools give the B matmuls 6 rotating banks.
 - in-place elementwise chains cut SBUF working-set and copies are merged
   (4x[128,128] transpose blocks -> one 3D-AP copy).
adaln params stay in the exact 3-pass split-bf16 scheme (fp32-accurate);
the computation is numerically chaotic (~1e-2 output sensitivity to any
fp32 reordering) so all math is kept bit-identical to the baseline.
"""
import functools
import numpy as np
import ml_dtypes

import concourse.bass as bass
import concourse.bacc as bacc_mod
import concourse.mybir as mybir
import concourse.tile as tile
from concourse.masks import make_identity
from concourse.bass_utils import run_bass_kernel_spmd

BF16 = ml_dtypes.bfloat16
FP8 = ml_dtypes.float8_e4m3fn
F32 = mybir.dt.float32
BF = mybir.dt.bfloat16
F8 = mybir.dt.float8e4
U32 = mybir.dt.uint32
AL = mybir.AluOpType
AF = mybir.ActivationFunctionType
AX = mybir.AxisListType

B, T, D = 4, 2048, 1024
TOK = 1024          # tokens per core
NH, HD = 16, 64
MLP = 4096
N_CORES = 8
C_MAGIC = float(1.5 * 2 ** 23)
MAGIC_U32 = 0x5F3759DF


def _quant_w(w):
    invws = float(np.clip(np.abs(w).mean(dtype=np.float64), 1e-5, None))
    m = np.clip(np.round(w.astype(np.float64) / invws), -1, 1).astype(np.float32)
    return m, np.float32(invws)


def _rsqrt(nc, sb, x_ap, scale, bias, shape, tag):
    """out = rsqrt(x*scale + bias), Newton on DVE. Returns a new tile."""
    t = sb.tile(shape, F32, tag=tag + "_t", name=tag + "_t")
    nc.vector.tensor_scalar(out=t, in0=x_ap, scalar1=float(scale),
                            scalar2=float(bias), op0=AL.mult, op1=AL.add)
    y = sb.tile(shape, F32, tag=tag + "_y", name=tag + "_y")
    sh = sb.tile(shape, F32, tag=tag + "_s", name=tag + "_s")
    nc.vector.tensor_scalar(out=sh[:].bitcast(U32), in0=t[:].bitcast(U32),
                            scalar1=1, scalar2=None, op0=AL.logical_shift_right)
    mg = sb.tile(shape, F32, tag=tag + "_m", name=tag + "_m")
    nc.vector.memset(mg[:].bitcast(U32), MAGIC_U32)
    nc.vector.tensor_tensor(out=y[:].bitcast(U32), in0=mg[:].bitcast(U32),
                            in1=sh[:].bitcast(U32), op=AL.subtract)
    e = sb.tile(shape, F32, tag=tag + "_e", name=tag + "_e")
    for _ in range(3):
        nc.vector.tensor_tensor(out=e, in0=y, in1=y, op=AL.mult)
        nc.vector.tensor_tensor(out=e, in0=e, in1=t, op=AL.mult)
        nc.vector.tensor_scalar(out=e, in0=e, scalar1=-0.5, scalar2=1.5,
                                op0=AL.mult, op1=AL.add)
        nc.vector.tensor_tensor(out=y, in0=y, in1=e, op=AL.mult)
    return y


def _build(iw):
    """iw: dict of invws floats. Returns finalized Bacc program."""
    nc = bacc_mod.Bacc("TRN2", target_bir_lowering=False)

    x_sl = nc.declare_dram_parameter("x_sl", [TOK, D], F32, isOutput=False)
    c_col = nc.declare_dram_parameter("c_col", [128, 8], F32, isOutput=False)
    adw_hi = nc.declare_dram_parameter("adw_hi", [12, 128, 8, 512], BF,
                                       isOutput=False)
    adw_lo = nc.declare_dram_parameter("adw_lo", [12, 128, 8, 512], BF,
                                       isOutput=False)
    adb_row = nc.declare_dram_parameter("adb_row", [12, 1, 512], F32,
                                        isOutput=False)
    mask8 = nc.declare_dram_parameter("mask8", [2, 1], F32, isOutput=False)
    gnr = nc.declare_dram_parameter("gnr", [1, D], F32, isOutput=False)
    wi4 = nc.declare_dram_parameter("wi4", [8, 128, 8, 128], F8, isOutput=False)
    wf4 = nc.declare_dram_parameter("wf4", [8, 128, 8, 128], F8, isOutput=False)
    wg3 = nc.declare_dram_parameter("wg3", [128, 8, D], F8, isOutput=False)
    wo3 = nc.declare_dram_parameter("wo3", [128, 8, D], F8, isOutput=False)
    gw4 = nc.declare_dram_parameter("gw4", [8, 128, 8, 1024], F8, isOutput=False)
    dw3 = nc.declare_dram_parameter("dw3", [128, 32, D], F8, isOutput=False)
    out_sl = nc.declare_dram_parameter("out_sl", [TOK, D], F32, isOutput=True)

    cc2_in = nc.dram_tensor("cc2_in", [D], F32)
    cc2_out = nc.dram_tensor("cc2_out", [2, D], F32)
    dqrow_d = nc.dram_tensor("dqrow_d", [D], F32)
    xnew_d = nc.dram_tensor("xnew_d", [TOK, D], F32)

    RG = [[2 * b, 2 * b + 1] for b in range(4)]

    with tile.TileContext(nc) as tc:
        # ---------- persistent pools ----------
        cst = tc.alloc_tile_pool(name="cst", bufs=1)
        big = tc.alloc_tile_pool(name="big", bufs=1)
        # right-side persistent broadcast pools (closed in LIFO as phases end)
        pG2 = tc.alloc_tile_pool(name="pG2", bufs=1, side="right")
        pSh2 = tc.alloc_tile_pool(name="pSh2", bufs=1, side="right")
        pG1 = tc.alloc_tile_pool(name="pG1", bufs=1, side="right")

        def bigt(shape, dtype, name):
            # 4 rotating 32KB slots; creation order == phase order:
            #  s0: x_res -> cam  -> xn_all -> dw_sb
            #  s1: moda  -> gs   -> x2qT
            #  s2: xqT   -> hT   -> h2a
            #  s3: ha    -> oqT  -> h2b
            return big.tile(shape, dtype, tag="bigslot", name=name, bufs=4)

        # constants (small)
        identb = cst.tile([128, 128], BF)
        make_identity(nc, identb)
        identf = cst.tile([128, 128], F32)
        make_identity(nc, identf)
        ones_row = cst.tile([1, 128], F32)
        nc.vector.memset(ones_row, 1.0)
        mask_sb = cst.tile([2, 1], F32)
        nc.sync.dma_start(out=mask_sb, in_=mask8[:, :])
        negC = cst.tile([128, 1], F32)
        nc.vector.memset(negC, -C_MAGIC)
        posC = cst.tile([128, 1], F32)
        nc.vector.memset(posC, C_MAGIC)
        q127A = cst.tile([128, 8], F32); dqA = cst.tile([128, 8], F32)
        dqAg = cst.tile([128, 8], F32)
        q127O = cst.tile([128, 8], F32); dqOo = cst.tile([128, 8], F32)
        q127C = cst.tile([128, 8], F32); dqCg = cst.tile([128, 8], F32)
        cs_hi = cst.tile([128, 8], BF); cs_lo = cst.tile([128, 8], BF)

        B_g2 = pG2.tile([128, D], F32)
        B_sh2 = pSh2.tile([128, D], F32)
        B_sc2 = pSh2.tile([128, D], F32)
        B_g1 = pG1.tile([128, D], F32)

        x_res = bigt([128, 8, D], F32, "x_res")      # s0
        moda = bigt([128, 8, D], F32, "moda")        # s1
        xqT = bigt([128, 8, D], BF, "xqT")           # s2
        ha = bigt([128, 8, D], F32, "ha")            # s3

        # ---------- shared psum-phase machinery ----------
        ps_holder = {}

        def ps_open(name, mm_bufs=4, tp_bufs=2, tpf_bufs=0, scr=False):
            p = tc.alloc_tile_pool(name=name, bufs=1, space="PSUM")
            ps_holder["p"] = p
            ps_holder["mm_bufs"] = mm_bufs
            ps_holder["tp_bufs"] = tp_bufs
            ps_holder["tpf_bufs"] = tpf_bufs
            ps_holder["scr"] = scr
            return p

        def ps_close():
            ps_holder["p"].release()

        def pmm(shape=(128, 512)):
            return ps_holder["p"].tile(list(shape), F32, tag="mm", name="mm",
                                       bufs=ps_holder["mm_bufs"])

        def ptp():
            return ps_holder["p"].tile([128, 512], BF, tag="tp", name="tp",
                                       bufs=ps_holder["tp_bufs"])

        def ptpf():
            return ps_holder["p"].tile([128, 512], F32, tag="tpf", name="tpf",
                                       bufs=ps_holder["tpf_bufs"])

        def pscr():
            return ps_holder["p"].tile([128, 512], F32, tag="scr", name="scr",
                                       bufs=1)

        # ---------- helpers ----------
        def quant_batch(amx, ssx, n, dk, q127, dqt, iws_scaled, sb_p, tagp):
            """q127 = 127/max(amx,1e-5); dq = amc*rsqrt(ssx/dk+1e-8)*s/127."""
            shape = [128, n]
            amc = sb_p.tile(shape, F32, tag=tagp + "amc", name=tagp + "amc")
            nc.vector.tensor_scalar(out=amc, in0=amx, scalar1=1e-5,
                                    scalar2=None, op0=AL.max)
            rs = _rsqrt(nc, sb_p, ssx, 1.0 / dk, 1e-8, shape, tagp + "rs")
            rec = sb_p.tile(shape, F32, tag=tagp + "rec", name=tagp + "rec")
            nc.vector.reciprocal(out=rec, in_=amc)
            nc.vector.tensor_scalar(out=q127, in0=rec, scalar1=127.0,
                                    scalar2=None, op0=AL.mult)
            dqv = sb_p.tile(shape, F32, tag=tagp + "dqv", name=tagp + "dqv")
            nc.vector.tensor_tensor(out=dqv, in0=amc, in1=rs, op=AL.mult)
            sc = (float(iws_scaled) if iws_scaled is not None else 1.0) / 127.0
            nc.vector.tensor_scalar(out=dqt, in0=dqv, scalar1=sc,
                                    scalar2=None, op0=AL.mult)

        def round_and_transpose(src, q_col, dst_bf, i, nblk, sb_p, tagp,
                                flip=0, kq_bufs=2):
            """round src [128, 128*nblk] -> bf16, transpose 128-blocks into
            dst_bf[:, j, 128i:...]. DVE/ACT roles alternate with `flip`;
            PSUM->SBUF copies are merged 4-blocks-at-a-time via 3D APs."""
            for ci, c0 in enumerate(range(0, nblk, 8)):
                nb8 = min(8, nblk - c0)
                w = 128 * nb8
                sv = src[:, 128 * c0:128 * c0 + w]
                t2 = sb_p.tile([128, 1024], F32,
                               bufs=(1 if tagp == "rc" else 2),
                               tag=tagp + "t2", name=tagp + "t2")
                kq = sb_p.tile([128, 1024], BF, bufs=kq_bufs,
                               tag=tagp + "kq", name=tagp + "kq")
                if (ci + flip) % 2 == 0:
                    nc.vector.tensor_scalar(out=t2[:, 0:w], in0=sv,
                                            scalar1=q_col, scalar2=C_MAGIC,
                                            op0=AL.mult, op1=AL.add)
                    nc.scalar.activation(out=kq[:, 0:w], in_=t2[:, 0:w],
                                         func=AF.Identity, bias=negC)
                else:
                    nc.scalar.activation(out=t2[:, 0:w], in_=sv,
                                         func=AF.Identity, scale=q_col,
                                         bias=posC)
                    nc.vector.tensor_scalar(out=kq[:, 0:w], in0=t2[:, 0:w],
                                            scalar1=-C_MAGIC, scalar2=None,
                                            op0=AL.add)
                for g4 in range(0, nb8, 4):
                    nb = min(4, nb8 - g4)
                    tp = ptp()
                    for jj in range(nb):
                        nc.tensor.transpose(
                            tp[:, 128 * jj:128 * (jj + 1)],
                            kq[:, 128 * (g4 + jj):128 * (g4 + jj + 1)],
                            identb)
                    dst = dst_bf[:, c0 + g4:c0 + g4 + nb,
                                 128 * i:128 * (i + 1)]
                    if (ci + g4 // 4 + flip) % 2 == 0:
                        nc.scalar.copy(out=dst, in_=tp[:, 0:128 * nb])
                    else:
                        nc.vector.tensor_copy(out=dst, in_=tp[:, 0:128 * nb])

        # chunk ch -> destination broadcast tile slice
        bdst = {}

        def adaln_chunks(ch_list, wk):
            # params = cs_hi@Whi + cs_hi@Wlo + cs_lo@Whi  (+bias)
            for ch in ch_list:
                adwh_c = wk.tile([128, 8, 512], BF, tag="adwh", bufs=2)
                nc.sync.dma_start(out=adwh_c, in_=adw_hi[ch])
                adwl_c = wk.tile([128, 8, 512], BF, tag="adwl", bufs=1)
                nc.sync.dma_start(out=adwl_c, in_=adw_lo[ch])
                adb_c = wk.tile([1, 512], F32, tag="adbc", bufs=1)
                nc.sync.dma_start(out=adb_c, in_=adb_row[ch])
                pa_ps = pmm((1, 512))
                for j in range(8):
                    nc.tensor.matmul(pa_ps, cs_hi[:, j:j + 1],
                                     adwh_c[:, j, :],
                                     start=(j == 0), stop=False)
                for j in range(8):
                    nc.tensor.matmul(pa_ps, cs_hi[:, j:j + 1],
                                     adwl_c[:, j, :], start=False, stop=False)
                for j in range(8):
                    nc.tensor.matmul(pa_ps, cs_lo[:, j:j + 1],
                                     adwh_c[:, j, :],
                                     start=False, stop=(j == 7))
                prow = wk.tile([1, 512], F32, tag="prow", bufs=1)
                nc.vector.tensor_tensor(out=prow, in0=pa_ps, in1=adb_c,
                                        op=AL.add)
                dst, off, plus1 = bdst[ch]
                pb_ps = pmm()
                nc.tensor.matmul(pb_ps, ones_row, prow, start=True, stop=True)
                if plus1:
                    nc.scalar.activation(out=dst[:, off:off + 512],
                                         in_=pb_ps, func=AF.Identity,
                                         bias=1.0)
                else:
                    nc.scalar.copy(out=dst[:, off:off + 512], in_=pb_ps)

        # =================== phase 0 + A ===============================
        pSb = tc.alloc_tile_pool(name="pSb", bufs=1)
        Sb_i = pSb.tile([128, D], F32)
        Sb_f = pSb.tile([128, D], F32)
        pShSc = tc.alloc_tile_pool(name="pShSc", bufs=1)
        B_sh1 = pShSc.tile([128, D], F32)
        B_sc1 = pShSc.tile([128, D], F32)

        bdst.update({0: (B_sh1, 0, False), 1: (B_sh1, 512, False),
                     2: (B_sc1, 0, True), 3: (B_sc1, 512, True),
                     4: (B_g1, 0, False), 5: (B_g1, 512, False),
                     6: (B_sh2, 0, False), 7: (B_sh2, 512, False),
                     8: (B_sc2, 0, True), 9: (B_sc2, 512, True),
                     10: (B_g2, 0, False), 11: (B_g2, 512, False)})

        ps_open("psA", mm_bufs=4, tp_bufs=2)
        wk = tc.alloc_tile_pool(name="p0", bufs=2)

        c_sb = wk.tile([128, 8], F32, tag="csb", bufs=1)
        nc.sync.dma_start(out=c_sb, in_=c_col[:, :])
        cs_sb = wk.tile([128, 8], F32, tag="cssb", bufs=1)
        nc.scalar.activation(out=cs_sb, in_=c_sb, func=AF.Silu)
        nc.vector.tensor_copy(out=cs_hi, in_=cs_sb)
        cs_hif = wk.tile([128, 8], F32, tag="cshif", bufs=1)
        nc.vector.tensor_copy(out=cs_hif, in_=cs_hi)
        nc.vector.tensor_tensor(out=cs_lo, in0=cs_sb, in1=cs_hif,
                                op=AL.subtract)

        # shift_msa/scale_msa first so modulate can start early
        adaln_chunks([0], wk)
        # resident x: emitted after chunk 0's weight loads so the first
        # adaln matmuls aren't blocked behind this 4MB DMA
        for i in range(8):
            nc.sync.dma_start(
                out=x_res[:, i, :],
                in_=x_sl[128 * i:128 * (i + 1), :])
        adaln_chunks([1, 2, 3], wk)

        # LN1 stats (pure DVE, overlaps adaln matmuls)
        muA = pShSc.tile([128, 8], F32)
        varA = pShSc.tile([128, 8], F32)
        for i in range(8):
            st = wk.tile([128, 2, 6], F32, tag="bst")
            xr = x_res[:, i, :].rearrange("p (s d) -> p s d", s=2)
            for s2 in range(2):
                nc.vector.bn_stats(out=st[:, s2, :], in_=xr[:, s2, :])
            mv = wk.tile([128, 2], F32, tag="bmv")
            nc.vector.bn_aggr(out=mv, in_=st)
            nc.vector.tensor_copy(out=muA[:, i:i + 1], in_=mv[:, 0:1])
            nc.vector.tensor_copy(out=varA[:, i:i + 1], in_=mv[:, 1:2])
        rstdLN = _rsqrt(nc, pShSc, varA, 1.0, 1e-6, [128, 8], "rLN")
        nmr = pShSc.tile([128, 8], F32)
        nc.vector.tensor_tensor(out=nmr, in0=muA, in1=rstdLN, op=AL.mult)
        nc.vector.tensor_scalar(out=nmr, in0=nmr, scalar1=-1.0,
                                scalar2=None, op0=AL.mult)

        # --- phase A: modulate + quant; adaln chunks 4..7 interleaved ---
        amA = pShSc.tile([128, 8], F32)
        ssA = pShSc.tile([128, 8], F32)
        ssdum = wk.tile([128, 1024], F32, tag="ssdum", bufs=1)
        for i in range(8):
            # moda = (x*rstd + nmr) * B_sc1 + B_sh1, in-place chain
            if i % 2 == 0:
                nc.scalar.activation(out=moda[:, i, :], in_=x_res[:, i, :],
                                     func=AF.Identity,
                                     scale=rstdLN[:, i:i + 1],
                                     bias=nmr[:, i:i + 1])
            else:
                nc.vector.tensor_scalar(out=moda[:, i, :], in0=x_res[:, i, :],
                                        scalar1=rstdLN[:, i:i + 1],
                                        scalar2=nmr[:, i:i + 1],
                                        op0=AL.mult, op1=AL.add)
            nc.vector.tensor_tensor(out=moda[:, i, :], in0=moda[:, i, :],
                                    in1=B_sc1, op=AL.mult)
            nc.gpsimd.tensor_tensor(out=moda[:, i, :], in0=moda[:, i, :],
                                    in1=B_sh1, op=AL.add)
            nc.vector.tensor_reduce(out=amA[:, i:i + 1], in_=moda[:, i, :],
                                    axis=AX.X, op=AL.max,
                                    apply_absolute_value=True)
            nc.scalar.activation(out=ssdum, in_=moda[:, i, :],
                                 func=AF.Square, accum_out=ssA[:, i:i + 1])
            if i % 2 == 1:
                adaln_chunks([4 + i // 2], wk)
        quant_batch(amA, ssA, 8, D, q127A, dqA, None, wk, "qa")
        nc.vector.tensor_scalar(out=dqAg, in0=dqA, scalar1=float(iw["g"]),
                                scalar2=None, op0=AL.mult)
        # dq row via DRAM bounce, then Sb_i / Sb_f broadcasts
        nc.sync.dma_start(out=dqrow_d[:].rearrange("(i p) -> p i", p=128),
                          in_=dqA)
        oi = wk.tile([1, 128], F32, tag="oi", bufs=1, name="oi")
        nc.vector.memset(oi, float(iw["i"]))
        of = wk.tile([1, 128], F32, tag="of", bufs=1, name="of")
        nc.vector.memset(of, float(iw["f"]))
        for i in range(8):
            round_and_transpose(moda[:, i, :], q127A[:, i:i + 1], xqT,
                                i, 8, wk, "ra", flip=i)
        for ch in range(0, D, 512):
            dqc = wk.tile([1, 512], F32, tag="adbc", name="dqc", bufs=1)
            nc.sync.dma_start(
                out=dqc,
                in_=dqrow_d[ch:ch + 512].rearrange("(one d) -> one d", one=1))
            pb_ps = pmm()
            nc.tensor.matmul(pb_ps, oi, dqc, start=True, stop=True)
            nc.scalar.copy(out=Sb_i[:, ch:ch + 512], in_=pb_ps)
            pb2 = pmm()
            nc.tensor.matmul(pb2, of, dqc, start=True, stop=True)
            nc.vector.tensor_copy(out=Sb_f[:, ch:ch + 512], in_=pb2)
        wk.release()
        pShSc.release()
        ps_close()

        # =================== phase B: i/f matmuls + scans ===============
        cam = bigt([128, 8, D], F32, "cam")          # s0
        ps_open("psB", mm_bufs=8, tp_bufs=0)
        pWg = tc.alloc_tile_pool(name="pWg", bufs=1, side="right")
        wg_sb = pWg.tile([128, 8, D], F8, tag="wgsb", bufs=1)
        nc.sync.dma_start(out=wg_sb, in_=wg3[:, :, :])
        pGn = tc.alloc_tile_pool(name="pGn", bufs=1, side="right")
        B_gn = pGn.tile([128, D], F32)
        pb = tc.alloc_tile_pool(name="pb", bufs=2)
        for m in range(8):
            wf_m = pb.tile([128, 8, 128], F8, tag="wfm")
            nc.sync.dma_start(out=wf_m, in_=wf4[m])
            wi_m = pb.tile([128, 8, 128], F8, tag="wim")
            nc.sync.dma_start(out=wi_m, in_=wi4[m])
            sigf_m = pb.tile([128, TOK], F32, tag="sigf", bufs=2)
            ifin_m = pb.tile([128, TOK], F32, tag="ifin", bufs=2)
            for ck in range(0, TOK, 512):
                pf = pmm()
                for j in range(8):
                    nc.tensor.matmul(pf, wf_m[:, j, :],
                                     xqT[:, j, ck:ck + 512],
                                     start=(j == 0), stop=(j == 7))
                pi = pmm()
                for j in range(8):
                    nc.tensor.matmul(pi, wi_m[:, j, :],
                                     xqT[:, j, ck:ck + 512],
                                     start=(j == 0), stop=(j == 7))
                ft = pb.tile([128, 512], F32, tag="ftm", bufs=2)
                it = pb.tile([128, 512], F32, tag="itm", bufs=2)
                nc.vector.tensor_tensor(out=ft, in0=pf,
                                        in1=Sb_f[:, ck:ck + 512], op=AL.mult)
                nc.vector.tensor_tensor(out=it, in0=pi,
                                        in1=Sb_i[:, ck:ck + 512], op=AL.mult)
                nc.scalar.activation(out=sigf_m[:, ck:ck + 512], in_=ft,
                                     func=AF.Sigmoid)
                omf = pb.tile([128, 512], F32, tag="omf", bufs=2)
                nc.scalar.activation(out=omf, in_=ft, func=AF.Sigmoid,
                                     scale=-1.0)
                sgi = pb.tile([128, 512], F32, tag="sgi", bufs=2)
                nc.scalar.activation(out=sgi, in_=it, func=AF.Sigmoid)
                sili = pb.tile([128, 512], F32, tag="sili", bufs=2)
                nc.gpsimd.tensor_tensor(out=sili, in0=it, in1=sgi,
                                        op=AL.mult)
                nc.gpsimd.tensor_tensor(out=ifin_m[:, ck:ck + 512],
                                        in0=sili, in1=omf, op=AL.mult)
            nc.vector.tensor_tensor_scan(ha[:, m, :], sigf_m, ifin_m, 0.0,
                                         op0=AL.mult, op1=AL.add)
            nc.vector.tensor_tensor_scan(cam[:, m, :], sigf_m, sigf_m, 1.0,
                                         op0=AL.mult, op1=AL.bypass)
            nc.sync.dma_start(
                out=cc2_in[128 * m:128 * (m + 1)].rearrange(
                    "(p one) -> p one", one=1),
                in_=ha[:, m, TOK - 1:TOK])
        for ch in range(0, D, 512):
            gnc = pb.tile([1, 512], F32, tag="gnc", name="gnc", bufs=1)
            nc.sync.dma_start(out=gnc, in_=gnr[:, ch:ch + 512])
            pb_ps = pmm()
            nc.tensor.matmul(pb_ps, ones_row, gnc, start=True, stop=True)
            nc.scalar.copy(out=B_gn[:, ch:ch + 512], in_=pb_ps)
        nc.gpsimd.collective_compute(
            "AllGather", AL.bypass, ins=[cc2_in[:]], outs=[cc2_out[:]],
            replica_groups=RG)
        pb.release()
        pSb.release()

        # ====== AllGather window: g-projection + adaln chunks 8..11 =====
        gs = bigt([128, 8, D], F32, "gs")            # s1
        pw_pool = tc.alloc_tile_pool(name="pwin", bufs=2)
        win_chunks = [8, 9, 10, 11]
        for m in range(8):
            for ck in range(0, D, 512):
                pg = pmm()
                for j in range(8):
                    nc.tensor.matmul(pg, xqT[:, j, 128 * m:128 * (m + 1)],
                                     wg_sb[:, j, ck:ck + 512],
                                     start=(j == 0), stop=(j == 7))
                sg2 = pw_pool.tile([128, 512], F32, tag="sg2", bufs=2)
                nc.scalar.activation(out=sg2, in_=pg, func=AF.Sigmoid,
                                     scale=dqAg[:, m:m + 1])
                xgn = pw_pool.tile([128, 512], F32, tag="xgn", bufs=2)
                nc.vector.scalar_tensor_tensor(
                    out=xgn, in0=pg, scalar=dqAg[:, m:m + 1],
                    in1=B_gn[:, ck:ck + 512], op0=AL.mult, op1=AL.mult)
                nc.vector.tensor_tensor(out=gs[:, m, ck:ck + 512],
                                        in0=xgn, in1=sg2, op=AL.mult)
            if m < 4:
                adaln_chunks([win_chunks[m]], pw_pool)
        pw_pool.release()
        ps_close()

        # ---------------- fixup: apply carry, transpose h ---------------
        hT = bigt([128, 8, D], F32, "hT")            # s2
        ps_open("psC", mm_bufs=4, tp_bufs=1, tpf_bufs=2, scr=True)
        pf_pool = tc.alloc_tile_pool(name="pf", bufs=2)
        ag2 = pf_pool.tile([2, D], F32, tag="ag2", bufs=1)
        nc.sync.dma_start(out=ag2, in_=cc2_out[:, :])
        for m in range(8):
            pc = pmm((128, 1))
            nc.tensor.matmul(pc, ag2[:, 128 * m:128 * (m + 1)], mask_sb,
                             start=True, stop=True)
            carry = pf_pool.tile([128, 1], F32, tag="carry")
            nc.scalar.copy(out=carry, in_=pc)
            hfix = pf_pool.tile([128, TOK], F32, tag="hfix", bufs=2)
            nc.vector.scalar_tensor_tensor(out=hfix, in0=cam[:, m, :],
                                           scalar=carry, in1=ha[:, m, :],
                                           op0=AL.mult, op1=AL.add)
            for g4 in range(0, 8, 4):
                tp = ptpf()
                for jj in range(4):
                    t_i = g4 + jj
                    nc.tensor.transpose(
                        tp[:, 128 * jj:128 * (jj + 1)],
                        hfix[:, 128 * t_i:128 * (t_i + 1)], identf)
                dst = hT[:, g4:g4 + 4, 128 * m:128 * (m + 1)]
                if (m + g4 // 4) % 2 == 0:
                    nc.scalar.copy(out=dst, in_=tp[:, 0:512])
                else:
                    nc.vector.tensor_copy(out=dst, in_=tp[:, 0:512])
        pf_pool.release()
        pGn.release()
        pWg.release()

        # ======== o-stage + phase C, software-pipelined per t ==========
        oqT = bigt([128, 8, D], BF, "oqT")           # s3
        xn_all = bigt([128, 8, D], F32, "xn_all")    # s0
        pM2 = tc.alloc_tile_pool(name="pM2", bufs=2)
        po = tc.alloc_tile_pool(name="po", bufs=2)
        wo_sb = po.tile([128, 8, D], F8, tag="wosb", bufs=1)
        nc.sync.dma_start(out=wo_sb, in_=wo3[:, :, :])
        mshA = po.tile([128, 8, 16], F32, tag="msh", bufs=1)
        muC = po.tile([128, 8], F32, tag="muC", bufs=1)
        varC = po.tile([128, 8], F32, tag="varC", bufs=1)
        amO = po.tile([128, 8], F32, tag="amO", bufs=1)
        ssO = po.tile([128, 8], F32, tag="ssO", bufs=1)
        ssdum2 = po.tile([128, 1024], F32, tag="ssdum2", bufs=1)
        rH_half = [None, None]
        xr2_tiles = {}

        def o_stats(t):
            sq = po.tile([128, D], F32, tag="sqo", bufs=1)
            nc.scalar.activation(out=sq, in_=hT[:, t, :], func=AF.Square)
            nc.vector.tensor_reduce(
                out=mshA[:, t, :],
                in_=sq.rearrange("p (h d) -> p h d", h=NH),
                axis=AX.X, op=AL.add)

        def o_gate(t):
            rH = rH_half[t // 4]
            rb = bass.AP(tensor=rH.tensor,
                         offset=rH[:, t % 4, :].offset,
                         ap=[rH.ap[0], [1, NH], [0, HD]])
            # hn in-place into hT; oa in-place into gs
            nc.vector.tensor_tensor(
                out=hT[:, t, :].rearrange("p (h d) -> p h d", h=NH),
                in0=hT[:, t, :].rearrange("p (h d) -> p h d", h=NH),
                in1=rb, op=AL.mult)
            if t % 2 == 0:
                nc.gpsimd.tensor_tensor(out=gs[:, t, :], in0=hT[:, t, :],
                                        in1=gs[:, t, :], op=AL.mult)
            else:
                nc.vector.tensor_tensor(out=gs[:, t, :], in0=hT[:, t, :],
                                        in1=gs[:, t, :], op=AL.mult)
            nc.vector.tensor_reduce(out=amO[:, t:t + 1], in_=gs[:, t, :],
                                    axis=AX.X, op=AL.max,
                                    apply_absolute_value=True)
            nc.scalar.activation(out=ssdum2, in_=gs[:, t, :],
                                 func=AF.Square, accum_out=ssO[:, t:t + 1])

        def xr2_prefetch(t):
            for cki in range(2):
                xt = po.tile([128, 512], F32, tag="xr2", bufs=4)
                nc.sync.dma_start(
                    out=xt,
                    in_=x_sl[128 * t:128 * (t + 1), 512 * cki:512 * (cki + 1)])
                xr2_tiles[(t, cki)] = xt

        def c_stage(t):
            st = po.tile([128, 2, 6], F32, tag="bst2")
            for cki, ck in enumerate(range(0, D, 512)):
                pw = pmm()
                for j in range(8):
                    nc.tensor.matmul(pw, oqT[:, j, 128 * t:128 * (t + 1)],
                                     wo_sb[:, j, ck:ck + 512],
                                     start=(j == 0), stop=(j == 7))
                v = po.tile([128, 512], F32, tag="vC", bufs=2)
                nc.vector.scalar_tensor_tensor(
                    out=v, in0=pw, scalar=dqOo[:, t:t + 1],
                    in1=B_g1[:, ck:ck + 512], op0=AL.mult, op1=AL.mult)
                if (t + cki) % 2 == 0:
                    nc.gpsimd.tensor_tensor(out=xn_all[:, t, ck:ck + 512],
                                            in0=v, in1=xr2_tiles[(t, cki)],
                                            op=AL.add)
                else:
                    nc.vector.tensor_tensor(out=xn_all[:, t, ck:ck + 512],
                                            in0=v, in1=xr2_tiles[(t, cki)],
                                            op=AL.add)
                nc.vector.bn_stats(out=st[:, cki, :],
                                   in_=xn_all[:, t, ck:ck + 512])
            nc.sync.dma_start(out=xnew_d[128 * t:128 * (t + 1), :],
                              in_=xn_all[:, t, :])
            mv = po.tile([128, 2], F32, tag="bmv2")
            nc.vector.bn_aggr(out=mv, in_=st)
            nc.vector.tensor_copy(out=muC[:, t:t + 1], in_=mv[:, 0:1])
            nc.vector.tensor_copy(out=varC[:, t:t + 1], in_=mv[:, 1:2])

        # ---- LN2 + modulate2 + quant, pipelined into phase D ----------
        x2qT = bigt([128, 8, D], BF, "x2qT")         # s1
        dw_sb = bigt([128, 32, D], F8, "dw_sb")      # s2 (hT dead)
        gw_first = pM2.tile([128, 8, 1024], F8, tag="gwf", bufs=1)
        nc.sync.dma_start(out=gw_first, in_=gw4[0])
        amC = pM2.tile([128, 8], F32, tag="amC", bufs=1)
        ssC = pM2.tile([128, 8], F32, tag="ssC", bufs=1)

        def mod2_elem(t, hf):
            rstdC, nmrC = rstd_nmr_C[hf]
            tl = t % 4
            # in-place chain on xn_all (D reads the xnew_d bounce instead)
            nc.scalar.activation(out=xn_all[:, t, :], in_=xn_all[:, t, :],
                                 func=AF.Identity,
                                 scale=rstdC[:, tl:tl + 1],
                                 bias=nmrC[:, tl:tl + 1])
            if t % 2 == 0:
                nc.gpsimd.tensor_tensor(out=xn_all[:, t, :],
                                        in0=xn_all[:, t, :],
                                        in1=B_sc2, op=AL.mult)
                nc.vector.tensor_tensor(out=xn_all[:, t, :],
                                        in0=xn_all[:, t, :],
                                        in1=B_sh2, op=AL.add)
            else:
                nc.vector.tensor_tensor(out=xn_all[:, t, :],
                                        in0=xn_all[:, t, :],
                                        in1=B_sc2, op=AL.mult)
                nc.gpsimd.tensor_tensor(out=xn_all[:, t, :],
                                        in0=xn_all[:, t, :],
                                        in1=B_sh2, op=AL.add)
        def mod2_quant(t):
            nc.vector.tensor_reduce(out=amC[:, t:t + 1], in_=xn_all[:, t, :],
                                    axis=AX.X, op=AL.max,
                                    apply_absolute_value=True)
            nc.scalar.activation(out=ssdum2, in_=xn_all[:, t, :],
                                 func=AF.Square, accum_out=ssC[:, t:t + 1])
            quant_batch(amC[:, t:t + 1], ssC[:, t:t + 1], 1, D,
                        q127C[:, t:t + 1], dqCg[:, t:t + 1],
                        iw["gate"], pM2, "qc")

        rstd_nmr_C = {}

        def rstdC_half(hf):
            t0 = 4 * hf
            rstdC = _rsqrt(nc, pM2, varC[:, t0:t0 + 4], 1.0, 1e-6,
                           [128, 4], "rC%d" % hf)
            nmrC = pM2.tile([128, 4], F32, tag="nmrC%d" % hf, bufs=1)
            nc.vector.tensor_tensor(out=nmrC, in0=muC[:, t0:t0 + 4],
                                    in1=rstdC, op=AL.mult)
            nc.vector.tensor_scalar(out=nmrC, in0=nmrC, scalar1=-1.0,
                                    scalar2=None, op0=AL.mult)
            rstd_nmr_C[hf] = (rstdC, nmrC)

        def o_chain(t):
            o_gate(t)
            quant_batch(amO[:, t:t + 1], ssO[:, t:t + 1], 1, D,
                        q127O[:, t:t + 1], dqOo[:, t:t + 1],
                        iw["o"], po, "qo")
            round_and_transpose(gs[:, t, :], q127O[:, t:t + 1], oqT,
                                t, 8, pM2, "rc", flip=t, kq_bufs=4)
            xr2_prefetch(t)

        for t in range(4):
            o_stats(t)
        rH_half[0] = _rsqrt(
            nc, po, mshA[:, 0:4, :].rearrange("p a b -> p (a b)"),
            1.0 / HD, 1e-5, [128, 64], "rH0")
        rH_half[0] = rH_half[0].rearrange("p (a b) -> p a b", a=4)
        for t in range(0, 4):
            if t > 0:
                c_stage(t - 1)
            o_chain(t)
            if t > 0:
                o_stats(3 + t)
        o_stats(7)
        rH_half[1] = _rsqrt(
            nc, po, mshA[:, 4:8, :].rearrange("p a b -> p (a b)"),
            1.0 / HD, 1e-5, [128, 64], "rH1")
        rH_half[1] = rH_half[1].rearrange("p (a b) -> p a b", a=4)
        for t in range(4, 8):
            c_stage(t - 1)
            if t == 4:
                rstdC_half(0)
            o_chain(t)
            mod2_elem(t - 4, 0)
        c_stage(7)

        # ---- mod2 quant+rounds (h0 rounds here; h1 under gate h0) ----
        for t in range(0, 4):
            mod2_quant(t)
            round_and_transpose(xn_all[:, t, :], q127C[:, t:t + 1],
                                x2qT, t, 8, pM2, "rc", flip=t,
                                kq_bufs=4)
        rstdC_half(1)
        for t in range(4, 8):
            mod2_elem(t, 1)
            mod2_quant(t)
        # t4..7 rounds: DVE/ACT part now; transposes deferred into gate-h0
        deferred = []
        for t in range(4, 8):
            ci = 0
            w = 1024
            sv = xn_all[:, t, :]
            t2 = pM2.tile([128, 1024], F32, bufs=1, tag="rct2")
            kq = pM2.tile([128, 1024], BF, bufs=4, tag="rckq")
            if t % 2 == 0:
                nc.vector.tensor_scalar(out=t2, in0=sv,
                                        scalar1=q127C[:, t:t + 1],
                                        scalar2=C_MAGIC,
                                        op0=AL.mult, op1=AL.add)
                nc.scalar.activation(out=kq, in_=t2, func=AF.Identity,
                                     bias=negC)
            else:
                nc.scalar.activation(out=t2, in_=sv, func=AF.Identity,
                                     scale=q127C[:, t:t + 1], bias=posC)
                nc.vector.tensor_scalar(out=kq, in0=t2, scalar1=-C_MAGIC,
                                        scalar2=None, op0=AL.add)
            deferred.append((t, kq))
        po.release()
        ps_close()
        pG1.release()
        pSh2.release()

        # =================== phase D: MLP ==============================
        h2a = bigt([128, 2, MLP], F32, "h2a")        # s3
        h2b = bigt([128, 2, MLP], F32, "h2b")        # s0
        ps_open("psD", mm_bufs=5, tp_bufs=2, scr=True)
        pde = tc.alloc_tile_pool(name="pde", bufs=2)

        def h2_of(ti):
            return h2a[:, ti, :] if ti < 2 else h2b[:, ti - 2, :]

        xn3_tiles = {}

        def xn3_prefetch(t):
            for cki in range(2):
                xt = pde.tile([128, 512], F32, tag="xn3", bufs=2)
                nc.sync.dma_start(
                    out=xt,
                    in_=xnew_d[128 * t:128 * (t + 1),
                               512 * cki:512 * (cki + 1)])
                xn3_tiles[(t, cki)] = xt

        for half in range(2):
            tof = 4 * half
            amDg = pde.tile([128, 4, 8], F32, tag="amDg", bufs=2)
            ssDg = pde.tile([128, 4, 8], F32, tag="ssDg", bufs=2)
            for g in range(8):
                if half == 0 and g == 0:
                    gw_g = gw_first
                else:
                    gw_g = pde.tile([128, 8, 1024], F8, tag="gwg", bufs=3)
                    nc.sync.dma_start(out=gw_g, in_=gw4[g])
                if half == 0 and g == 4:
                    nc.sync.dma_start(out=dw_sb, in_=dw3[:, :, :])
                if half == 0 and 1 <= g <= 4:
                    # deferred x2qT transposes for t=4..7 ride the gate-h0
                    # PE stream (their DVE/ACT rounds are long since done)
                    t, kq = deferred[g - 1]
                    for g4 in range(0, 8, 4):
                        tp = ptp()
                        for jj in range(4):
                            nc.tensor.transpose(
                                tp[:, 128 * jj:128 * (jj + 1)],
                                kq[:, 128 * (g4 + jj):128 * (g4 + jj + 1)],
                                identb)
                        dst = x2qT[:, g4:g4 + 4, 128 * t:128 * (t + 1)]
                        if (t + g4 // 4) % 2 == 0:
                            nc.scalar.copy(out=dst, in_=tp[:, 0:512])
                        else:
                            nc.vector.tensor_copy(out=dst, in_=tp[:, 0:512])
                for ti in range(4):
                    t = tof + ti
                    pgg = pmm()
                    for j in range(8):
                        nc.tensor.matmul(
                            pgg, x2qT[:, j, 128 * t:128 * (t + 1)],
                            gw_g[:, j, 0:512],
                            start=(j == 0), stop=(j == 7))
                    pyy = pmm()
                    for j in range(8):
                        nc.tensor.matmul(
                            pyy, x2qT[:, j, 128 * t:128 * (t + 1)],
                            gw_g[:, j, 512:1024],
                            start=(j == 0), stop=(j == 7))
                    sil = pde.tile([128, 512], F32, tag="sil", bufs=2)
                    nc.scalar.activation(out=sil, in_=pgg, func=AF.Silu,
                                         scale=dqCg[:, t:t + 1])
                    h2c = h2_of(ti)[:, 512 * g:512 * (g + 1)]
                    nc.vector.scalar_tensor_tensor(
                        out=h2c, in0=pyy, scalar=dqCg[:, t:t + 1],
                        in1=sil, op0=AL.mult, op1=AL.mult)
                    nc.vector.tensor_reduce(
                        out=amDg[:, ti, g:g + 1], in_=h2c,
                        axis=AX.X, op=AL.max, apply_absolute_value=True)
                    scr = pscr()
                    nc.scalar.activation(
                        out=scr, in_=h2c,
                        func=AF.Square, accum_out=ssDg[:, ti, g:g + 1])
            amD = pde.tile([128, 4], F32, tag="amD", bufs=2)
            ssD = pde.tile([128, 4], F32, tag="ssD", bufs=2)
            nc.vector.tensor_reduce(out=amD, in_=amDg, axis=AX.X, op=AL.max)
            nc.vector.tensor_reduce(out=ssD, in_=ssDg, axis=AX.X, op=AL.add)
            q127h = pde.tile([128, 4], F32, tag="q127h", bufs=2)
            dqh = pde.tile([128, 4], F32, tag="dqh", bufs=2)
            quant_batch(amD, ssD, 4, MLP, q127h, dqh, None, pde, "qd")
            nc.vector.tensor_scalar(out=dqh, in0=dqh,
                                    scalar1=float(iw["down"]),
                                    scalar2=None, op0=AL.mult)
            for ti in range(4):
                t = tof + ti
                h2qT = pde.tile([128, 32, 128], BF, tag="h2qT", bufs=1)
                round_and_transpose(h2_of(ti), q127h[:, ti:ti + 1],
                                    h2qT, 0, 32, pM2, "rc", flip=ti,
                                    kq_bufs=4)
                xn3_prefetch(t)
                for cki, ck in enumerate(range(0, D, 512)):
                    pdn = pmm()
                    for j2 in range(32):
                        nc.tensor.matmul(
                            pdn, h2qT[:, j2, :],
                            dw_sb[:, j2, ck:ck + 512],
                            start=(j2 == 0), stop=(j2 == 31))
                    v2 = pde.tile([128, 512], F32, tag="v2d", bufs=2)
                    nc.vector.scalar_tensor_tensor(
                        out=v2, in0=pdn, scalar=dqh[:, ti:ti + 1],
                        in1=B_g2[:, ck:ck + 512],
                        op0=AL.mult, op1=AL.mult)
                    nc.gpsimd.tensor_tensor(out=v2, in0=v2,
                                            in1=xn3_tiles[(t, cki)],
                                            op=AL.add)
                    nc.sync.dma_start(
                        out=out_sl[128 * t:128 * (t + 1), ck:ck + 512],
                        in_=v2)
        pde.release()
        ps_close()
        pM2.release()
        pG2.release()
        big.release()
        cst.release()

    nc.finalize()
    return nc


@functools.lru_cache(maxsize=2)
def _build_cached(iw_items):
    return _build(dict(iw_items))


def kernel(x, c, adaln_w, adaln_b, wi, wf, wg, gnorm_w, wo, gate_w, down_w):
    x = np.ascontiguousarray(np.asarray(x, dtype=np.float32))
    c = np.ascontiguousarray(np.asarray(c, dtype=np.float32))
    adaln_w = np.asarray(adaln_w, dtype=np.float32)
    adaln_b = np.asarray(adaln_b, dtype=np.float32)
    gnorm_w = np.asarray(gnorm_w, dtype=np.float32)

    mi, iwi = _quant_w(np.asarray(wi, dtype=np.float32))
    mf, iwf = _quant_w(np.asarray(wf, dtype=np.float32))
    mg, iwg = _quant_w(np.asarray(wg, dtype=np.float32))
    mo, iwo = _quant_w(np.asarray(wo, dtype=np.float32))
    mgate, iwgate = _quant_w(np.asarray(gate_w, dtype=np.float32))
    mdown, iwdown = _quant_w(np.asarray(down_w, dtype=np.float32))

    iw = {"i": float(iwi), "f": float(iwf), "g": float(iwg), "o": float(iwo),
          "gate": float(iwgate), "down": float(iwdown)}
    nc = _build_cached(tuple(sorted(iw.items())))

    # device layouts (see _build for index conventions)
    def stat4(w):   # [8(m), 128(p), 8(j), 128(q)]; w[oc, c]
        return np.ascontiguousarray(
            w.reshape(8, 128, 8, 128).transpose(0, 3, 2, 1).astype(FP8))

    def mov3(w):    # [128(p), 8(j), OC]; w[oc, c]
        return np.ascontiguousarray(
            w.T.reshape(8, 128, -1).transpose(1, 0, 2).astype(FP8))

    wi4_h = stat4(mi)
    wf4_h = stat4(mf)
    wg3_h = mov3(mg)
    wo3_h = mov3(mo)
    gA = mgate[:MLP].reshape(8, 512, 8, 128)     # [g, mc, j, p]
    gB = mgate[MLP:].reshape(8, 512, 8, 128)
    gw4_h = np.ascontiguousarray(np.concatenate(
        [gA.transpose(0, 3, 2, 1), gB.transpose(0, 3, 2, 1)],
        axis=3).astype(FP8))                      # [8, 128p, 8j, 1024]
    dw3_h = np.ascontiguousarray(
        mdown.T.reshape(32, 128, D).transpose(1, 0, 2).astype(FP8))

    adwT = adaln_w.T                              # [D(c), 6D(o)]
    adw_hi_f = adwT.astype(BF16).astype(np.float32)
    adw_lo_f = adwT - adw_hi_f

    def adw4(wf32):   # [12(ch), 128(p), 8(j), 512]
        return np.ascontiguousarray(
            wf32.reshape(8, 128, 12, 512).transpose(2, 1, 0, 3).astype(BF16))

    adw_hi_h = adw4(adw_hi_f)
    adw_lo_h = adw4(adw_lo_f)
    adb_row_h = np.ascontiguousarray(adaln_b.reshape(12, 1, 512))
    gnr_h = np.ascontiguousarray(np.tile(gnorm_w, NH)[None, :])

    in_maps = []
    for core in range(N_CORES):
        b, half = core // 2, core % 2
        mask = np.zeros((2, 1), np.float32)
        if half == 1:
            mask[0, 0] = 1.0
        c_col_h = np.ascontiguousarray(c[b].reshape(8, 128).T)   # [128(p), 8(j)]
        in_maps.append({
            "x_sl": np.ascontiguousarray(x[b, half * TOK:(half + 1) * TOK, :]),
            "c_col": c_col_h,
            "adw_hi": adw_hi_h,
            "adw_lo": adw_lo_h,
            "adb_row": adb_row_h,
            "mask8": mask,
            "gnr": gnr_h,
            "wi4": wi4_h, "wf4": wf4_h, "wg3": wg3_h, "wo3": wo3_h,
            "gw4": gw4_h, "dw3": dw3_h,
        })

    res = run_bass_kernel_spmd(nc, in_maps, core_ids=list(range(N_CORES)))
    out = np.zeros((B, T, D), np.float32)
    for core in range(N_CORES):
        b, half = core // 2, core % 2
        out[b, half * TOK:(half + 1) * TOK, :] = res.results[core]["out_sl"]
    return out


# revision 20
# speedup vs baseline: 1.0029x; 1.0029x over previous
"""Trainium2 Bass kernel for nn_DiTBlock (HGRN-attention DiT block).

Sharding: 8 cores = 4 batches x 2 half-sequences (1024 tokens each).
All bitlinear matmuls are exact integer arithmetic: activations quantized
to int8-range integers stored in bf16, ternary weights stored in fp8e4
(exact, half the DMA of bf16). The time recurrence h_t = f_t*h_{t-1} + i_t
runs on the DVE tensor_tensor_scan; the half-sequence boundary carry
crosses cores via one AllGather issued right after the last scan and
hidden under the g-projection matmuls + late adaln chunks.

Schedule (vs the phase-sequential baseline):
 - g-projection + adaln chunks 8..11 moved into the AllGather window.
 - o-stage / wo-matmul / LN2 / MLP are software-pipelined per token-block
   so PE matmuls overlap the DVE/ACT quant chains.
 - GpSimd (Pool engine) takes SBUF-only elementwise work (silu muls in
   the scan prep, modulate adds, residual adds) off the DVE.
 - Per-phase PSUM pools give the B matmuls 6 rotating banks.
 - in-place elementwise chains cut SBUF working-set and copies are merged
   (4x[128,128] transpose blocks -> one 3D-AP copy).
adaln params stay in the exact 3-pass split-bf16 scheme (fp32-accurate);
the computation is numerically chaotic (~1e-2 output sensitivity to any
fp32 reordering) so all math is kept bit-identical to the baseline.
"""
import functools
import numpy as np
import ml_dtypes

import concourse.bass as bass
import concourse.bacc as bacc_mod
import concourse.mybir as mybir
import concourse.tile as tile
from concourse.masks import make_identity
from concourse.bass_utils import run_bass_kernel_spmd

BF16 = ml_dtypes.bfloat16
FP8 = ml_dtypes.float8_e4m3fn
F32 = mybir.dt.float32
BF = mybir.dt.bfloat16
F8 = mybir.dt.float8e4
U32 = mybir.dt.uint32
AL = mybir.AluOpType
AF = mybir.ActivationFunctionType
AX = mybir.AxisListType

B, T, D = 4, 2048, 1024
TOK = 1024          # tokens per core
NH, HD = 16, 64
MLP = 4096
N_CORES = 8
C_MAGIC = float(1.5 * 2 ** 23)
MAGIC_U32 = 0x5F3759DF


def _quant_w(w):
    invws = float(np.clip(np.abs(w).mean(dtype=np.float64), 1e-5, None))
    m = np.clip(np.round(w.astype(np.float64) / invws), -1, 1).astype(np.float32)
    return m, np.float32(invws)


def _rsqrt(nc, sb, x_ap, scale, bias, shape, tag):
    """out = rsqrt(x*scale + bias), Newton on DVE. Returns a new tile."""
    t = sb.tile(shape, F32, tag=tag + "_t", name=tag + "_t")
    nc.vector.tensor_scalar(out=t, in0=x_ap, scalar1=float(scale),
                            scalar2=float(bias), op0=AL.mult, op1=AL.add)
    y = sb.tile(shape, F32, tag=tag + "_y", name=tag + "_y")
    sh = sb.tile(shape, F32, tag=tag + "_s", name=tag + "_s")
    nc.vector.tensor_scalar(out=sh[:].bitcast(U32), in0=t[:].bitcast(U32),
                            scalar1=1, scalar2=None, op0=AL.logical_shift_right)
    mg = sb.tile(shape, F32, tag=tag + "_m", name=tag + "_m")
    nc.vector.memset(mg[:].bitcast(U32), MAGIC_U32)
    nc.vector.tensor_tensor(out=y[:].bitcast(U32), in0=mg[:].bitcast(U32),
                            in1=sh[:].bitcast(U32), op=AL.subtract)
    e = sb.tile(shape, F32, tag=tag + "_e", name=tag + "_e")
    for _ in range(3):
        nc.vector.tensor_tensor(out=e, in0=y, in1=y, op=AL.mult)
        nc.vector.tensor_tensor(out=e, in0=e, in1=t, op=AL.mult)
        nc.vector.tensor_scalar(out=e, in0=e, scalar1=-0.5, scalar2=1.5,
                                op0=AL.mult, op1=AL.add)
        nc.vector.tensor_tensor(out=y, in0=y, in1=e, op=AL.mult)
    return y


def _build(iw):
    """iw: dict of invws floats. Returns finalized Bacc program."""
    nc = bacc_mod.Bacc("TRN2", target_bir_lowering=False)

    x_sl = nc.declare_dram_parameter("x_sl", [TOK, D], F32, isOutput=False)
    c_col = nc.declare_dram_parameter("c_col", [128, 8], F32, isOutput=False)
    adw_hi = nc.declare_dram_parameter("adw_hi", [12, 128, 8, 512], BF,
                                       isOutput=False)
    adw_lo = nc.declare_dram_parameter("adw_lo", [12, 128, 8, 512], BF,
                                       isOutput=False)
    adb_row = nc.declare_dram_parameter("adb_row", [12, 1, 512], F32,
                                        isOutput=False)
    mask8 = nc.declare_dram_parameter("mask8", [2, 1], F32, isOutput=False)
    gnr = nc.declare_dram_parameter("gnr", [1, D], F32, isOutput=False)
    wi4 = nc.declare_dram_parameter("wi4", [8, 128, 8, 128], F8, isOutput=False)
    wf4 = nc.declare_dram_parameter("wf4", [8, 128, 8, 128], F8, isOutput=False)
    wg3 = nc.declare_dram_parameter("wg3", [128, 8, D], F8, isOutput=False)
    wo3 = nc.declare_dram_parameter("wo3", [128, 8, D], F8, isOutput=False)
    gw4 = nc.declare_dram_parameter("gw4", [8, 128, 8, 1024], F8, isOutput=False)
    dw3 = nc.declare_dram_parameter("dw3", [128, 32, D], F8, isOutput=False)
    out_sl = nc.declare_dram_parameter("out_sl", [TOK, D], F32, isOutput=True)

    cc2_in = nc.dram_tensor("cc2_in", [D], F32)
    cc2_out = nc.dram_tensor("cc2_out", [2, D], F32)
    dqrow_d = nc.dram_tensor("dqrow_d", [D], F32)
    xnew_d = nc.dram_tensor("xnew_d", [TOK, D], F32)

    RG = [[2 * b, 2 * b + 1] for b in range(4)]

    with tile.TileContext(nc) as tc:
        # ---------- persistent pools ----------
        cst = tc.alloc_tile_pool(name="cst", bufs=1)
        big = tc.alloc_tile_pool(name="big", bufs=1)
        # right-side persistent broadcast pools (closed in LIFO as phases end)
        pG2 = tc.alloc_tile_pool(name="pG2", bufs=1, side="right")
        pSh2 = tc.alloc_tile_pool(name="pSh2", bufs=1, side="right")
        pG1 = tc.alloc_tile_pool(name="pG1", bufs=1, side="right")

        def bigt(shape, dtype, name):
            # 4 rotating 32KB slots; creation order == phase order:
            #  s0: x_res -> cam  -> xn_all -> dw_sb
            #  s1: moda  -> gs   -> x2qT
            #  s2: xqT   -> hT   -> h2a
            #  s3: ha    -> oqT  -> h2b
            return big.tile(shape, dtype, tag="bigslot", name=name, bufs=4)

        # constants (small)
        identb = cst.tile([128, 128], BF)
        make_identity(nc, identb)
        identf = cst.tile([128, 128], F32)
        make_identity(nc, identf)
        ones_row = cst.tile([1, 128], F32)
        nc.vector.memset(ones_row, 1.0)
        mask_sb = cst.tile([2, 1], F32)
        nc.sync.dma_start(out=mask_sb, in_=mask8[:, :])
        negC = cst.tile([128, 1], F32)
        nc.vector.memset(negC, -C_MAGIC)
        posC = cst.tile([128, 1], F32)
        nc.vector.memset(posC, C_MAGIC)
        q127A = cst.tile([128, 8], F32); dqA = cst.tile([128, 8], F32)
        dqAg = cst.tile([128, 8], F32)
        q127O = cst.tile([128, 8], F32); dqOo = cst.tile([128, 8], F32)
        q127C = cst.tile([128, 8], F32); dqCg = cst.tile([128, 8], F32)
        cs_hi = cst.tile([128, 8], BF); cs_lo = cst.tile([128, 8], BF)

        B_g2 = pG2.tile([128, D], F32)
        B_sh2 = pSh2.tile([128, D], F32)
        B_sc2 = pSh2.tile([128, D], F32)
        B_g1 = pG1.tile([128, D], F32)

        x_res = bigt([128, 8, D], F32, "x_res")      # s0
        moda = bigt([128, 8, D], F32, "moda")        # s1
        xqT = bigt([128, 8, D], BF, "xqT")           # s2
        ha = bigt([128, 8, D], F32, "ha")            # s3

        # ---------- shared psum-phase machinery ----------
        ps_holder = {}

        def ps_open(name, mm_bufs=4, tp_bufs=2, tpf_bufs=0, scr=False):
            p = tc.alloc_tile_pool(name=name, bufs=1, space="PSUM")
            ps_holder["p"] = p
            ps_holder["mm_bufs"] = mm_bufs
            ps_holder["tp_bufs"] = tp_bufs
            ps_holder["tpf_bufs"] = tpf_bufs
            ps_holder["scr"] = scr
            return p

        def ps_close():
            ps_holder["p"].release()

        def pmm(shape=(128, 512)):
            return ps_holder["p"].tile(list(shape), F32, tag="mm", name="mm",
                                       bufs=ps_holder["mm_bufs"])

        def ptp():
            return ps_holder["p"].tile([128, 512], BF, tag="tpx", name="tp",
                                       bufs=ps_holder["tp_bufs"])

        def ptpf():
            return ps_holder["p"].tile([128, 512], F32, tag="tpx", name="tpf",
                                       bufs=ps_holder["tp_bufs"])

        def pscr():
            return ps_holder["p"].tile([128, 512], F32, tag="scr", name="scr",
                                       bufs=1)

        # ---------- helpers ----------
        def quant_batch(amx, ssx, n, dk, q127, dqt, iws_scaled, sb_p, tagp):
            """q127 = 127/max(amx,1e-5); dq = amc*rsqrt(ssx/dk+1e-8)*s/127."""
            shape = [128, n]
            amc = sb_p.tile(shape, F32, tag=tagp + "amc", name=tagp + "amc")
            nc.vector.tensor_scalar(out=amc, in0=amx, scalar1=1e-5,
                                    scalar2=None, op0=AL.max)
            rs = _rsqrt(nc, sb_p, ssx, 1.0 / dk, 1e-8, shape, tagp + "rs")
            rec = sb_p.tile(shape, F32, tag=tagp + "rec", name=tagp + "rec")
            nc.vector.reciprocal(out=rec, in_=amc)
            nc.vector.tensor_scalar(out=q127, in0=rec, scalar1=127.0,
                                    scalar2=None, op0=AL.mult)
            dqv = sb_p.tile(shape, F32, tag=tagp + "dqv", name=tagp + "dqv")
            nc.vector.tensor_tensor(out=dqv, in0=amc, in1=rs, op=AL.mult)
            sc = (float(iws_scaled) if iws_scaled is not None else 1.0) / 127.0
            nc.vector.tensor_scalar(out=dqt, in0=dqv, scalar1=sc,
                                    scalar2=None, op0=AL.mult)

        def round_and_transpose(src, q_col, dst_bf, i, nblk, sb_p, tagp,
                                flip=0, kq_bufs=2):
            """round src [128, 128*nblk] -> bf16, transpose 128-blocks into
            dst_bf[:, j, 128i:...]. DVE/ACT roles alternate with `flip`;
            PSUM->SBUF copies are merged 4-blocks-at-a-time via 3D APs."""
            for ci, c0 in enumerate(range(0, nblk, 8)):
                nb8 = min(8, nblk - c0)
                w = 128 * nb8
                sv = src[:, 128 * c0:128 * c0 + w]
                t2 = sb_p.tile([128, 1024], F32,
                               bufs=(1 if tagp == "rc" else 2),
                               tag=tagp + "t2", name=tagp + "t2")
                kq = sb_p.tile([128, 1024], BF, bufs=kq_bufs,
                               tag=tagp + "kq", name=tagp + "kq")
                if (ci + flip) % 2 == 0:
                    nc.vector.tensor_scalar(out=t2[:, 0:w], in0=sv,
                                            scalar1=q_col, scalar2=C_MAGIC,
                                            op0=AL.mult, op1=AL.add)
                    nc.scalar.activation(out=kq[:, 0:w], in_=t2[:, 0:w],
                                         func=AF.Identity, bias=negC)
                else:
                    nc.scalar.activation(out=t2[:, 0:w], in_=sv,
                                         func=AF.Identity, scale=q_col,
                                         bias=posC)
                    nc.vector.tensor_scalar(out=kq[:, 0:w], in0=t2[:, 0:w],
                                            scalar1=-C_MAGIC, scalar2=None,
                                            op0=AL.add)
                for g4 in range(0, nb8, 4):
                    nb = min(4, nb8 - g4)
                    tp = ptp()
                    for jj in range(nb):
                        nc.tensor.transpose(
                            tp[:, 128 * jj:128 * (jj + 1)],
                            kq[:, 128 * (g4 + jj):128 * (g4 + jj + 1)],
                            identb)
                    dst = dst_bf[:, c0 + g4:c0 + g4 + nb,
                                 128 * i:128 * (i + 1)]
                    if (ci + g4 // 4 + flip) % 2 == 0:
                        nc.scalar.copy(out=dst, in_=tp[:, 0:128 * nb])
                    else:
                        nc.vector.tensor_copy(out=dst, in_=tp[:, 0:128 * nb])

        # chunk ch -> destination broadcast tile slice
        bdst = {}

        def adaln_chunks(ch_list, wk):
            # params = cs_hi@Whi + cs_hi@Wlo + cs_lo@Whi  (+bias)
            for ch in ch_list:
                adwh_c = wk.tile([128, 8, 512], BF, tag="adwh", bufs=2)
                nc.sync.dma_start(out=adwh_c, in_=adw_hi[ch])
                adwl_c = wk.tile([128, 8, 512], BF, tag="adwl", bufs=1)
                nc.sync.dma_start(out=adwl_c, in_=adw_lo[ch])
                adb_c = wk.tile([1, 512], F32, tag="adbc", bufs=1)
                nc.sync.dma_start(out=adb_c, in_=adb_row[ch])
                pa_ps = pmm((1, 512))
                for j in range(8):
                    nc.tensor.matmul(pa_ps, cs_hi[:, j:j + 1],
                                     adwh_c[:, j, :],
                                     start=(j == 0), stop=False)
                for j in range(8):
                    nc.tensor.matmul(pa_ps, cs_hi[:, j:j + 1],
                                     adwl_c[:, j, :], start=False, stop=False)
                for j in range(8):
                    nc.tensor.matmul(pa_ps, cs_lo[:, j:j + 1],
                                     adwh_c[:, j, :],
                                     start=False, stop=(j == 7))
                prow = wk.tile([1, 512], F32, tag="prow", bufs=1)
                nc.vector.tensor_tensor(out=prow, in0=pa_ps, in1=adb_c,
                                        op=AL.add)
                dst, off, plus1 = bdst[ch]
                pb_ps = pmm()
                nc.tensor.matmul(pb_ps, ones_row, prow, start=True, stop=True)
                if plus1:
                    nc.scalar.activation(out=dst[:, off:off + 512],
                                         in_=pb_ps, func=AF.Identity,
                                         bias=1.0)
                else:
                    nc.scalar.copy(out=dst[:, off:off + 512], in_=pb_ps)

        # =================== phase 0 + A ===============================
        pSb = tc.alloc_tile_pool(name="pSb", bufs=1)
        Sb_i = pSb.tile([128, D], F32)
        Sb_f = pSb.tile([128, D], F32)
        pShSc = tc.alloc_tile_pool(name="pShSc", bufs=1)
        B_sh1 = pShSc.tile([128, D], F32)
        B_sc1 = pShSc.tile([128, D], F32)

        bdst.update({0: (B_sh1, 0, False), 1: (B_sh1, 512, False),
                     2: (B_sc1, 0, True), 3: (B_sc1, 512, True),
                     4: (B_g1, 0, False), 5: (B_g1, 512, False),
                     6: (B_sh2, 0, False), 7: (B_sh2, 512, False),
                     8: (B_sc2, 0, True), 9: (B_sc2, 512, True),
                     10: (B_g2, 0, False), 11: (B_g2, 512, False)})

        ps_open("psA", mm_bufs=4, tp_bufs=2)
        wk = tc.alloc_tile_pool(name="p0", bufs=2)

        c_sb = wk.tile([128, 8], F32, tag="csb", bufs=1)
        nc.sync.dma_start(out=c_sb, in_=c_col[:, :])
        cs_sb = wk.tile([128, 8], F32, tag="cssb", bufs=1)
        nc.scalar.activation(out=cs_sb, in_=c_sb, func=AF.Silu)
        nc.vector.tensor_copy(out=cs_hi, in_=cs_sb)
        cs_hif = wk.tile([128, 8], F32, tag="cshif", bufs=1)
        nc.vector.tensor_copy(out=cs_hif, in_=cs_hi)
        nc.vector.tensor_tensor(out=cs_lo, in0=cs_sb, in1=cs_hif,
                                op=AL.subtract)

        # shift_msa/scale_msa first so modulate can start early
        adaln_chunks([0], wk)
        # resident x: emitted after chunk 0's weight loads so the first
        # adaln matmuls aren't blocked behind this 4MB DMA
        for i in range(8):
            nc.sync.dma_start(
                out=x_res[:, i, :],
                in_=x_sl[128 * i:128 * (i + 1), :])
        adaln_chunks([1, 2, 3], wk)

        # LN1 stats (pure DVE, overlaps adaln matmuls)
        muA = pShSc.tile([128, 8], F32)
        varA = pShSc.tile([128, 8], F32)
        for i in range(8):
            st = wk.tile([128, 2, 6], F32, tag="bst")
            xr = x_res[:, i, :].rearrange("p (s d) -> p s d", s=2)
            for s2 in range(2):
                nc.vector.bn_stats(out=st[:, s2, :], in_=xr[:, s2, :])
            mv = wk.tile([128, 2], F32, tag="bmv")
            nc.vector.bn_aggr(out=mv, in_=st)
            nc.vector.tensor_copy(out=muA[:, i:i + 1], in_=mv[:, 0:1])
            nc.vector.tensor_copy(out=varA[:, i:i + 1], in_=mv[:, 1:2])
        rstdLN = _rsqrt(nc, pShSc, varA, 1.0, 1e-6, [128, 8], "rLN")
        nmr = pShSc.tile([128, 8], F32)
        nc.vector.tensor_tensor(out=nmr, in0=muA, in1=rstdLN, op=AL.mult)
        nc.vector.tensor_scalar(out=nmr, in0=nmr, scalar1=-1.0,
                                scalar2=None, op0=AL.mult)

        # --- phase A: modulate + quant; adaln chunks 4..7 interleaved ---
        amA = pShSc.tile([128, 8], F32)
        ssA = pShSc.tile([128, 8], F32)
        ssdum = wk.tile([128, 1024], F32, tag="ssdum", bufs=1)
        for i in range(8):
            # moda = (x*rstd + nmr) * B_sc1 + B_sh1, in-place chain
            if i % 2 == 0:
                nc.scalar.activation(out=moda[:, i, :], in_=x_res[:, i, :],
                                     func=AF.Identity,
                                     scale=rstdLN[:, i:i + 1],
                                     bias=nmr[:, i:i + 1])
            else:
                nc.vector.tensor_scalar(out=moda[:, i, :], in0=x_res[:, i, :],
                                        scalar1=rstdLN[:, i:i + 1],
                                        scalar2=nmr[:, i:i + 1],
                                        op0=AL.mult, op1=AL.add)
            nc.vector.tensor_tensor(out=moda[:, i, :], in0=moda[:, i, :],
                                    in1=B_sc1, op=AL.mult)
            nc.gpsimd.tensor_tensor(out=moda[:, i, :], in0=moda[:, i, :],
                                    in1=B_sh1, op=AL.add)
            nc.vector.tensor_reduce(out=amA[:, i:i + 1], in_=moda[:, i, :],
                                    axis=AX.X, op=AL.max,
                                    apply_absolute_value=True)
            nc.scalar.activation(out=ssdum, in_=moda[:, i, :],
                                 func=AF.Square, accum_out=ssA[:, i:i + 1])
            if i % 2 == 1:
                adaln_chunks([4 + i // 2], wk)
        quant_batch(amA, ssA, 8, D, q127A, dqA, None, wk, "qa")
        nc.vector.tensor_scalar(out=dqAg, in0=dqA, scalar1=float(iw["g"]),
                                scalar2=None, op0=AL.mult)
        # dq row via DRAM bounce, then Sb_i / Sb_f broadcasts
        nc.sync.dma_start(out=dqrow_d[:].rearrange("(i p) -> p i", p=128),
                          in_=dqA)
        oi = wk.tile([1, 128], F32, tag="oi", bufs=1, name="oi")
        nc.vector.memset(oi, float(iw["i"]))
        of = wk.tile([1, 128], F32, tag="of", bufs=1, name="of")
        nc.vector.memset(of, float(iw["f"]))
        for i in range(8):
            round_and_transpose(moda[:, i, :], q127A[:, i:i + 1], xqT,
                                i, 8, wk, "ra", flip=i)
        for ch in range(0, D, 512):
            dqc = wk.tile([1, 512], F32, tag="adbc", name="dqc", bufs=1)
            nc.sync.dma_start(
                out=dqc,
                in_=dqrow_d[ch:ch + 512].rearrange("(one d) -> one d", one=1))
            pb_ps = pmm()
            nc.tensor.matmul(pb_ps, oi, dqc, start=True, stop=True)
            nc.scalar.copy(out=Sb_i[:, ch:ch + 512], in_=pb_ps)
            pb2 = pmm()
            nc.tensor.matmul(pb2, of, dqc, start=True, stop=True)
            nc.vector.tensor_copy(out=Sb_f[:, ch:ch + 512], in_=pb2)
        wk.release()
        pShSc.release()
        ps_close()

        # =================== phase B: i/f matmuls + scans ===============
        cam = bigt([128, 8, D], F32, "cam")          # s0
        ps_open("psB", mm_bufs=8, tp_bufs=0)
        pWg = tc.alloc_tile_pool(name="pWg", bufs=1, side="right")
        wg_sb = pWg.tile([128, 8, D], F8, tag="wgsb", bufs=1)
        nc.sync.dma_start(out=wg_sb, in_=wg3[:, :, :])
        pGn = tc.alloc_tile_pool(name="pGn", bufs=1, side="right")
        B_gn = pGn.tile([128, D], F32)
        pb = tc.alloc_tile_pool(name="pb", bufs=2)
        for m in range(8):
            wf_m = pb.tile([128, 8, 128], F8, tag="wfm")
            nc.sync.dma_start(out=wf_m, in_=wf4[m])
            wi_m = pb.tile([128, 8, 128], F8, tag="wim")
            nc.sync.dma_start(out=wi_m, in_=wi4[m])
            sigf_m = pb.tile([128, TOK], F32, tag="sigf", bufs=2)
            ifin_m = pb.tile([128, TOK], F32, tag="ifin", bufs=2)
            for ck in range(0, TOK, 512):
                pf = pmm()
                for j in range(8):
                    nc.tensor.matmul(pf, wf_m[:, j, :],
                                     xqT[:, j, ck:ck + 512],
                                     start=(j == 0), stop=(j == 7))
                pi = pmm()
                for j in range(8):
                    nc.tensor.matmul(pi, wi_m[:, j, :],
                                     xqT[:, j, ck:ck + 512],
                                     start=(j == 0), stop=(j == 7))
                ft = pb.tile([128, 512], F32, tag="ftm", bufs=2)
                it = pb.tile([128, 512], F32, tag="itm", bufs=2)
                nc.vector.tensor_tensor(out=ft, in0=pf,
                                        in1=Sb_f[:, ck:ck + 512], op=AL.mult)
                nc.vector.tensor_tensor(out=it, in0=pi,
                                        in1=Sb_i[:, ck:ck + 512], op=AL.mult)
                nc.scalar.activation(out=sigf_m[:, ck:ck + 512], in_=ft,
                                     func=AF.Sigmoid)
                omf = pb.tile([128, 512], F32, tag="omf", bufs=2)
                nc.scalar.activation(out=omf, in_=ft, func=AF.Sigmoid,
                                     scale=-1.0)
                sgi = pb.tile([128, 512], F32, tag="sgi", bufs=2)
                nc.scalar.activation(out=sgi, in_=it, func=AF.Sigmoid)
                sili = pb.tile([128, 512], F32, tag="sili", bufs=2)
                nc.gpsimd.tensor_tensor(out=sili, in0=it, in1=sgi,
                                        op=AL.mult)
                nc.gpsimd.tensor_tensor(out=ifin_m[:, ck:ck + 512],
                                        in0=sili, in1=omf, op=AL.mult)
            nc.vector.tensor_tensor_scan(ha[:, m, :], sigf_m, ifin_m, 0.0,
                                         op0=AL.mult, op1=AL.add)
            nc.vector.tensor_tensor_scan(cam[:, m, :], sigf_m, sigf_m, 1.0,
                                         op0=AL.mult, op1=AL.bypass)
            nc.sync.dma_start(
                out=cc2_in[128 * m:128 * (m + 1)].rearrange(
                    "(p one) -> p one", one=1),
                in_=ha[:, m, TOK - 1:TOK])
        for ch in range(0, D, 512):
            gnc = pb.tile([1, 512], F32, tag="gnc", name="gnc", bufs=1)
            nc.sync.dma_start(out=gnc, in_=gnr[:, ch:ch + 512])
            pb_ps = pmm()
            nc.tensor.matmul(pb_ps, ones_row, gnc, start=True, stop=True)
            nc.scalar.copy(out=B_gn[:, ch:ch + 512], in_=pb_ps)
        nc.gpsimd.collective_compute(
            "AllGather", AL.bypass, ins=[cc2_in[:]], outs=[cc2_out[:]],
            replica_groups=RG)
        pb.release()
        pSb.release()

        # ====== AllGather window: g-projection + adaln chunks 8..11 =====
        gs = bigt([128, 8, D], F32, "gs")            # s1
        pw_pool = tc.alloc_tile_pool(name="pwin", bufs=2)
        win_chunks = [8, 9, 10, 11]
        for m in range(8):
            for ck in range(0, D, 512):
                pg = pmm()
                for j in range(8):
                    nc.tensor.matmul(pg, xqT[:, j, 128 * m:128 * (m + 1)],
                                     wg_sb[:, j, ck:ck + 512],
                                     start=(j == 0), stop=(j == 7))
                sg2 = pw_pool.tile([128, 512], F32, tag="sg2", bufs=2)
                nc.scalar.activation(out=sg2, in_=pg, func=AF.Sigmoid,
                                     scale=dqAg[:, m:m + 1])
                xgn = pw_pool.tile([128, 512], F32, tag="xgn", bufs=2)
                nc.vector.scalar_tensor_tensor(
                    out=xgn, in0=pg, scalar=dqAg[:, m:m + 1],
                    in1=B_gn[:, ck:ck + 512], op0=AL.mult, op1=AL.mult)
                nc.vector.tensor_tensor(out=gs[:, m, ck:ck + 512],
                                        in0=xgn, in1=sg2, op=AL.mult)
            if m < 4:
                adaln_chunks([win_chunks[m]], pw_pool)
        pw_pool.release()
        ps_close()

        # ---------------- fixup: apply carry, transpose h ---------------
        hT = bigt([128, 8, D], F32, "hT")            # s2
        ps_open("psC", mm_bufs=4, tp_bufs=2, scr=True)
        pf_pool = tc.alloc_tile_pool(name="pf", bufs=2)
        ag2 = pf_pool.tile([2, D], F32, tag="ag2", bufs=1)
        nc.sync.dma_start(out=ag2, in_=cc2_out[:, :])
        for m in range(8):
            pc = pmm((128, 1))
            nc.tensor.matmul(pc, ag2[:, 128 * m:128 * (m + 1)], mask_sb,
                             start=True, stop=True)
            carry = pf_pool.tile([128, 1], F32, tag="carry")
            nc.scalar.copy(out=carry, in_=pc)
            hfix = pf_pool.tile([128, TOK], F32, tag="hfix", bufs=2)
            nc.vector.scalar_tensor_tensor(out=hfix, in0=cam[:, m, :],
                                           scalar=carry, in1=ha[:, m, :],
                                           op0=AL.mult, op1=AL.add)
            for g4 in range(0, 8, 4):
                tp = ptpf()
                for jj in range(4):
                    t_i = g4 + jj
                    nc.tensor.transpose(
                        tp[:, 128 * jj:128 * (jj + 1)],
                        hfix[:, 128 * t_i:128 * (t_i + 1)], identf)
                dst = hT[:, g4:g4 + 4, 128 * m:128 * (m + 1)]
                if (m + g4 // 4) % 2 == 0:
                    nc.scalar.copy(out=dst, in_=tp[:, 0:512])
                else:
                    nc.vector.tensor_copy(out=dst, in_=tp[:, 0:512])
        pf_pool.release()
        pGn.release()
        pWg.release()

        # ======== o-stage + phase C, software-pipelined per t ==========
        oqT = bigt([128, 8, D], BF, "oqT")           # s3
        xn_all = bigt([128, 8, D], F32, "xn_all")    # s0
        pM2 = tc.alloc_tile_pool(name="pM2", bufs=2)
        po = tc.alloc_tile_pool(name="po", bufs=2)
        wo_sb = po.tile([128, 8, D], F8, tag="wosb", bufs=1)
        nc.sync.dma_start(out=wo_sb, in_=wo3[:, :, :])
        mshA = po.tile([128, 8, 16], F32, tag="msh", bufs=1)
        muC = po.tile([128, 8], F32, tag="muC", bufs=1)
        varC = po.tile([128, 8], F32, tag="varC", bufs=1)
        amO = po.tile([128, 8], F32, tag="amO", bufs=1)
        ssO = po.tile([128, 8], F32, tag="ssO", bufs=1)
        ssdum2 = po.tile([128, 1024], F32, tag="ssdum2", bufs=1)
        rH_half = [None, None]
        xr2_tiles = {}

        def o_stats(t):
            sq = po.tile([128, D], F32, tag="sqo", bufs=1)
            nc.scalar.activation(out=sq, in_=hT[:, t, :], func=AF.Square)
            nc.vector.tensor_reduce(
                out=mshA[:, t, :],
                in_=sq.rearrange("p (h d) -> p h d", h=NH),
                axis=AX.X, op=AL.add)

        def o_gate(t):
            rH = rH_half[t // 4]
            rb = bass.AP(tensor=rH.tensor,
                         offset=rH[:, t % 4, :].offset,
                         ap=[rH.ap[0], [1, NH], [0, HD]])
            # hn in-place into hT; oa in-place into gs
            nc.vector.tensor_tensor(
                out=hT[:, t, :].rearrange("p (h d) -> p h d", h=NH),
                in0=hT[:, t, :].rearrange("p (h d) -> p h d", h=NH),
                in1=rb, op=AL.mult)
            if t % 2 == 0:
                nc.gpsimd.tensor_tensor(out=gs[:, t, :], in0=hT[:, t, :],
                                        in1=gs[:, t, :], op=AL.mult)
            else:
                nc.vector.tensor_tensor(out=gs[:, t, :], in0=hT[:, t, :],
                                        in1=gs[:, t, :], op=AL.mult)
            nc.vector.tensor_reduce(out=amO[:, t:t + 1], in_=gs[:, t, :],
                                    axis=AX.X, op=AL.max,
                                    apply_absolute_value=True)
            nc.scalar.activation(out=ssdum2, in_=gs[:, t, :],
                                 func=AF.Square, accum_out=ssO[:, t:t + 1])

        def xr2_prefetch(t):
            for cki in range(2):
                xt = po.tile([128, 512], F32, tag="xr2", bufs=4)
                nc.sync.dma_start(
                    out=xt,
                    in_=x_sl[128 * t:128 * (t + 1), 512 * cki:512 * (cki + 1)])
                xr2_tiles[(t, cki)] = xt

        def c_stage(t):
            st = po.tile([128, 2, 6], F32, tag="bst2")
            for cki, ck in enumerate(range(0, D, 512)):
                pw = pmm()
                for j in range(8):
                    nc.tensor.matmul(pw, oqT[:, j, 128 * t:128 * (t + 1)],
                                     wo_sb[:, j, ck:ck + 512],
                                     start=(j == 0), stop=(j == 7))
                v = po.tile([128, 512], F32, tag="vC", bufs=2)
                nc.vector.scalar_tensor_tensor(
                    out=v, in0=pw, scalar=dqOo[:, t:t + 1],
                    in1=B_g1[:, ck:ck + 512], op0=AL.mult, op1=AL.mult)
                if (t + cki) % 2 == 0:
                    nc.gpsimd.tensor_tensor(out=xn_all[:, t, ck:ck + 512],
                                            in0=v, in1=xr2_tiles[(t, cki)],
                                            op=AL.add)
                else:
                    nc.vector.tensor_tensor(out=xn_all[:, t, ck:ck + 512],
                                            in0=v, in1=xr2_tiles[(t, cki)],
                                            op=AL.add)
                nc.vector.bn_stats(out=st[:, cki, :],
                                   in_=xn_all[:, t, ck:ck + 512])
            nc.sync.dma_start(out=xnew_d[128 * t:128 * (t + 1), :],
                              in_=xn_all[:, t, :])
            mv = po.tile([128, 2], F32, tag="bmv2")
            nc.vector.bn_aggr(out=mv, in_=st)
            nc.vector.tensor_copy(out=muC[:, t:t + 1], in_=mv[:, 0:1])
            nc.vector.tensor_copy(out=varC[:, t:t + 1], in_=mv[:, 1:2])

        # ---- LN2 + modulate2 + quant, pipelined into phase D ----------
        x2qT = bigt([128, 8, D], BF, "x2qT")         # s1
        dw_sb = bigt([128, 32, D], F8, "dw_sb")      # s2 (hT dead)
        gw_first = pM2.tile([128, 8, 1024], F8, tag="gwf", bufs=1)
        nc.sync.dma_start(out=gw_first, in_=gw4[0])
        amC = pM2.tile([128, 8], F32, tag="amC", bufs=1)
        ssC = pM2.tile([128, 8], F32, tag="ssC", bufs=1)

        def mod2_elem(t, hf):
            rstdC, nmrC = rstd_nmr_C[hf]
            tl = t % 4
            # in-place chain on xn_all (D reads the xnew_d bounce instead)
            nc.scalar.activation(out=xn_all[:, t, :], in_=xn_all[:, t, :],
                                 func=AF.Identity,
                                 scale=rstdC[:, tl:tl + 1],
                                 bias=nmrC[:, tl:tl + 1])
            if t % 2 == 0:
                nc.gpsimd.tensor_tensor(out=xn_all[:, t, :],
                                        in0=xn_all[:, t, :],
                                        in1=B_sc2, op=AL.mult)
                nc.vector.tensor_tensor(out=xn_all[:, t, :],
                                        in0=xn_all[:, t, :],
                                        in1=B_sh2, op=AL.add)
            else:
                nc.vector.tensor_tensor(out=xn_all[:, t, :],
                                        in0=xn_all[:, t, :],
                                        in1=B_sc2, op=AL.mult)
                nc.gpsimd.tensor_tensor(out=xn_all[:, t, :],
                                        in0=xn_all[:, t, :],
                                        in1=B_sh2, op=AL.add)
        def mod2_quant(t):
            nc.vector.tensor_reduce(out=amC[:, t:t + 1], in_=xn_all[:, t, :],
                                    axis=AX.X, op=AL.max,
                                    apply_absolute_value=True)
            nc.scalar.activation(out=ssdum2, in_=xn_all[:, t, :],
                                 func=AF.Square, accum_out=ssC[:, t:t + 1])
            quant_batch(amC[:, t:t + 1], ssC[:, t:t + 1], 1, D,
                        q127C[:, t:t + 1], dqCg[:, t:t + 1],
                        iw["gate"], pM2, "qc")

        rstd_nmr_C = {}

        def rstdC_half(hf):
            t0 = 4 * hf
            rstdC = _rsqrt(nc, pM2, varC[:, t0:t0 + 4], 1.0, 1e-6,
                           [128, 4], "rC%d" % hf)
            nmrC = pM2.tile([128, 4], F32, tag="nmrC%d" % hf, bufs=1)
            nc.vector.tensor_tensor(out=nmrC, in0=muC[:, t0:t0 + 4],
                                    in1=rstdC, op=AL.mult)
            nc.vector.tensor_scalar(out=nmrC, in0=nmrC, scalar1=-1.0,
                                    scalar2=None, op0=AL.mult)
            rstd_nmr_C[hf] = (rstdC, nmrC)

        def o_chain(t):
            o_gate(t)
            quant_batch(amO[:, t:t + 1], ssO[:, t:t + 1], 1, D,
                        q127O[:, t:t + 1], dqOo[:, t:t + 1],
                        iw["o"], po, "qo")
            round_and_transpose(gs[:, t, :], q127O[:, t:t + 1], oqT,
                                t, 8, pM2, "rc", flip=t, kq_bufs=4)
            xr2_prefetch(t)

        for t in range(4):
            o_stats(t)
        rH_half[0] = _rsqrt(
            nc, po, mshA[:, 0:4, :].rearrange("p a b -> p (a b)"),
            1.0 / HD, 1e-5, [128, 64], "rH0")
        rH_half[0] = rH_half[0].rearrange("p (a b) -> p a b", a=4)
        for t in range(0, 4):
            if t > 0:
                c_stage(t - 1)
            o_chain(t)
            if t > 0:
                o_stats(3 + t)
        o_stats(7)
        rH_half[1] = _rsqrt(
            nc, po, mshA[:, 4:8, :].rearrange("p a b -> p (a b)"),
            1.0 / HD, 1e-5, [128, 64], "rH1")
        rH_half[1] = rH_half[1].rearrange("p (a b) -> p a b", a=4)
        for t in range(4, 8):
            c_stage(t - 1)
            if t == 4:
                rstdC_half(0)
            o_chain(t)
            mod2_elem(t - 4, 0)
        c_stage(7)

        # ---- mod2 quant+rounds (h0 rounds here; h1 under gate h0) ----
        for t in range(0, 4):
            mod2_quant(t)
            round_and_transpose(xn_all[:, t, :], q127C[:, t:t + 1],
                                x2qT, t, 8, pM2, "rc", flip=t,
                                kq_bufs=4)
        rstdC_half(1)
        for t in range(4, 8):
            mod2_elem(t, 1)
            mod2_quant(t)
        # t4..7 rounds: DVE/ACT part now; transposes deferred into gate-h0
        deferred = []
        for t in range(4, 8):
            ci = 0
            w = 1024
            sv = xn_all[:, t, :]
            t2 = pM2.tile([128, 1024], F32, bufs=1, tag="rct2")
            kq = pM2.tile([128, 1024], BF, bufs=4, tag="rckq")
            if t % 2 == 0:
                nc.vector.tensor_scalar(out=t2, in0=sv,
                                        scalar1=q127C[:, t:t + 1],
                                        scalar2=C_MAGIC,
                                        op0=AL.mult, op1=AL.add)
                nc.scalar.activation(out=kq, in_=t2, func=AF.Identity,
                                     bias=negC)
            else:
                nc.scalar.activation(out=t2, in_=sv, func=AF.Identity,
                                     scale=q127C[:, t:t + 1], bias=posC)
                nc.vector.tensor_scalar(out=kq, in0=t2, scalar1=-C_MAGIC,
                                        scalar2=None, op0=AL.add)
            deferred.append((t, kq))
        po.release()
        ps_close()
        pG1.release()
        pSh2.release()

        # =================== phase D: MLP ==============================
        h2a = bigt([128, 2, MLP], F32, "h2a")        # s3
        h2b = bigt([128, 2, MLP], F32, "h2b")        # s0
        ps_open("psD", mm_bufs=5, tp_bufs=2, scr=True)
        pde = tc.alloc_tile_pool(name="pde", bufs=2)

        def h2_of(ti):
            return h2a[:, ti, :] if ti < 2 else h2b[:, ti - 2, :]

        xn3_tiles = {}

        def xn3_prefetch(t):
            for cki in range(2):
                xt = pde.tile([128, 512], F32, tag="xn3", bufs=2)
                nc.sync.dma_start(
                    out=xt,
                    in_=xnew_d[128 * t:128 * (t + 1),
                               512 * cki:512 * (cki + 1)])
                xn3_tiles[(t, cki)] = xt

        for half in range(2):
            tof = 4 * half
            amDg = pde.tile([128, 4, 8], F32, tag="amDg", bufs=2)
            ssDg = pde.tile([128, 4, 8], F32, tag="ssDg", bufs=2)
            for g in range(8):
                if half == 0 and g == 0:
                    gw_g = gw_first
                else:
                    gw_g = pde.tile([128, 8, 1024], F8, tag="gwg", bufs=3)
                    nc.sync.dma_start(out=gw_g, in_=gw4[g])
                if half == 0 and g == 4:
                    nc.sync.dma_start(out=dw_sb, in_=dw3[:, :, :])
                if half == 0 and 1 <= g <= 4:
                    # deferred x2qT transposes for t=4..7 ride the gate-h0
                    # PE stream (their DVE/ACT rounds are long since done)
                    t, kq = deferred[g - 1]
                    for g4 in range(0, 8, 4):
                        tp = ptp()
                        for jj in range(4):
                            nc.tensor.transpose(
                                tp[:, 128 * jj:128 * (jj + 1)],
                                kq[:, 128 * (g4 + jj):128 * (g4 + jj + 1)],
                                identb)
                        dst = x2qT[:, g4:g4 + 4, 128 * t:128 * (t + 1)]
                        if (t + g4 // 4) % 2 == 0:
                            nc.scalar.copy(out=dst, in_=tp[:, 0:512])
                        else:
                            nc.vector.tensor_copy(out=dst, in_=tp[:, 0:512])
                for ti in range(4):
                    t = tof + ti
                    pgg = pmm()
                    for j in range(8):
                        nc.tensor.matmul(
                            pgg, x2qT[:, j, 128 * t:128 * (t + 1)],
                            gw_g[:, j, 0:512],
                            start=(j == 0), stop=(j == 7))
                    pyy = pmm()
                    for j in range(8):
                        nc.tensor.matmul(
                            pyy, x2qT[:, j, 128 * t:128 * (t + 1)],
                            gw_g[:, j, 512:1024],
                            start=(j == 0), stop=(j == 7))
                    sil = pde.tile([128, 512], F32, tag="sil", bufs=2)
                    nc.scalar.activation(out=sil, in_=pgg, func=AF.Silu,
                                         scale=dqCg[:, t:t + 1])
                    h2c = h2_of(ti)[:, 512 * g:512 * (g + 1)]
                    nc.vector.scalar_tensor_tensor(
                        out=h2c, in0=pyy, scalar=dqCg[:, t:t + 1],
                        in1=sil, op0=AL.mult, op1=AL.mult)
                    nc.vector.tensor_reduce(
                        out=amDg[:, ti, g:g + 1], in_=h2c,
                        axis=AX.X, op=AL.max, apply_absolute_value=True)
                    scr = pscr()
                    nc.scalar.activation(
                        out=scr, in_=h2c,
                        func=AF.Square, accum_out=ssDg[:, ti, g:g + 1])
            amD = pde.tile([128, 4], F32, tag="amD", bufs=2)
            ssD = pde.tile([128, 4], F32, tag="ssD", bufs=2)
            nc.vector.tensor_reduce(out=amD, in_=amDg, axis=AX.X, op=AL.max)
            nc.vector.tensor_reduce(out=ssD, in_=ssDg, axis=AX.X, op=AL.add)
            q127h = pde.tile([128, 4], F32, tag="q127h", bufs=2)
            dqh = pde.tile([128, 4], F32, tag="dqh", bufs=2)
            quant_batch(amD, ssD, 4, MLP, q127h, dqh, None, pde, "qd")
            nc.vector.tensor_scalar(out=dqh, in0=dqh,
                                    scalar1=float(iw["down"]),
                                    scalar2=None, op0=AL.mult)
            for ti in range(4):
                t = tof + ti
                h2qT = pde.tile([128, 32, 128], BF, tag="h2qT", bufs=1)
                round_and_transpose(h2_of(ti), q127h[:, ti:ti + 1],
                                    h2qT, 0, 32, pM2, "rc", flip=ti,
                                    kq_bufs=4)
                xn3_prefetch(t)
                for cki, ck in enumerate(range(0, D, 512)):
                    pdn = pmm()
                    for j2 in range(32):
                        nc.tensor.matmul(
                            pdn, h2qT[:, j2, :],
                            dw_sb[:, j2, ck:ck + 512],
                            start=(j2 == 0), stop=(j2 == 31))
                    v2 = pde.tile([128, 512], F32, tag="v2d", bufs=2)
                    nc.vector.scalar_tensor_tensor(
                        out=v2, in0=pdn, scalar=dqh[:, ti:ti + 1],
                        in1=B_g2[:, ck:ck + 512],
                        op0=AL.mult, op1=AL.mult)
                    nc.gpsimd.tensor_tensor(out=v2, in0=v2,
                                            in1=xn3_tiles[(t, cki)],
                                            op=AL.add)
                    nc.sync.dma_start(
                        out=out_sl[128 * t:128 * (t + 1), ck:ck + 512],
                        in_=v2)
        pde.release()
        ps_close()
        pM2.release()
        pG2.release()
        big.release()
        cst.release()

    nc.finalize()
    return nc


@functools.lru_cache(maxsize=2)
def _build_cached(iw_items):
    return _build(dict(iw_items))


def kernel(x, c, adaln_w, adaln_b, wi, wf, wg, gnorm_w, wo, gate_w, down_w):
    x = np.ascontiguousarray(np.asarray(x, dtype=np.float32))
    c = np.ascontiguousarray(np.asarray(c, dtype=np.float32))
    adaln_w = np.asarray(adaln_w, dtype=np.float32)
    adaln_b = np.asarray(adaln_b, dtype=np.float32)
    gnorm_w = np.asarray(gnorm_w, dtype=np.float32)

    mi, iwi = _quant_w(np.asarray(wi, dtype=np.float32))
    mf, iwf = _quant_w(np.asarray(wf, dtype=np.float32))
    mg, iwg = _quant_w(np.asarray(wg, dtype=np.float32))
    mo, iwo = _quant_w(np.asarray(wo, dtype=np.float32))
    mgate, iwgate = _quant_w(np.asarray(gate_w, dtype=np.float32))
    mdown, iwdown = _quant_w(np.asarray(down_w, dtype=np.float32))

    iw = {"i": float(iwi), "f": float(iwf), "g": float(iwg), "o": float(iwo),
          "gate": float(iwgate), "down": float(iwdown)}
    nc = _build_cached(tuple(sorted(iw.items())))

    # device layouts (see _build for index conventions)
    def stat4(w):   # [8(m), 128(p), 8(j), 128(q)]; w[oc, c]
        return np.ascontiguousarray(
            w.reshape(8, 128, 8, 128).transpose(0, 3, 2, 1).astype(FP8))

    def mov3(w):    # [128(p), 8(j), OC]; w[oc, c]
        return np.ascontiguousarray(
            w.T.reshape(8, 128, -1).transpose(1, 0, 2).astype(FP8))

    wi4_h = stat4(mi)
    wf4_h = stat4(mf)
    wg3_h = mov3(mg)
    wo3_h = mov3(mo)
    gA = mgate[:MLP].reshape(8, 512, 8, 128)     # [g, mc, j, p]
    gB = mgate[MLP:].reshape(8, 512, 8, 128)
    gw4_h = np.ascontiguousarray(np.concatenate(
        [gA.transpose(0, 3, 2, 1), gB.transpose(0, 3, 2, 1)],
        axis=3).astype(FP8))                      # [8, 128p, 8j, 1024]
    dw3_h = np.ascontiguousarray(
        mdown.T.reshape(32, 128, D).transpose(1, 0, 2).astype(FP8))

    adwT = adaln_w.T                              # [D(c), 6D(o)]
    adw_hi_f = adwT.astype(BF16).astype(np.float32)
    adw_lo_f = adwT - adw_hi_f

    def adw4(wf32):   # [12(ch), 128(p), 8(j), 512]
        return np.ascontiguousarray(
            wf32.reshape(8, 128, 12, 512).transpose(2, 1, 0, 3).astype(BF16))

    adw_hi_h = adw4(adw_hi_f)
    adw_lo_h = adw4(adw_lo_f)
    adb_row_h = np.ascontiguousarray(adaln_b.reshape(12, 1, 512))
    gnr_h = np.ascontiguousarray(np.tile(gnorm_w, NH)[None, :])

    in_maps = []
    for core in range(N_CORES):
        b, half = core // 2, core % 2
        mask = np.zeros((2, 1), np.float32)
        if half == 1:
            mask[0, 0] = 1.0
        c_col_h = np.ascontiguousarray(c[b].reshape(8, 128).T)   # [128(p), 8(j)]
        in_maps.append({
            "x_sl": np.ascontiguousarray(x[b, half * TOK:(half + 1) * TOK, :]),
            "c_col": c_col_h,
            "adw_hi": adw_hi_h,
            "adw_lo": adw_lo_h,
            "adb_row": adb_row_h,
            "mask8": mask,
            "gnr": gnr_h,
            "wi4": wi4_h, "wf4": wf4_h, "wg3": wg3_h, "wo3": wo3_h,
            "gw4": gw4_h, "dw3": dw3_h,
        })

    res = run_bass_kernel_spmd(nc, in_maps, core_ids=list(range(N_CORES)))
    out = np.zeros((B, T, D), np.float32)
    for core in range(N_CORES):
        b, half = core // 2, core % 2
        out[b, half * TOK:(half + 1) * TOK, :] = res.results[core]["out_sl"]
    return out
